# revision 1
# baseline (speedup 1.0000x reference)
"""GraphSAGE (3-layer, mean-agg) on 8 Trainium2 NeuronCores — v3.

Strategy (nodes sharded by id range, weights replicated, edges
partitioned by destination owner):
  - Each core's 6250 destinations are split into two ID-halves (chunk
    A = local ids [0,3125), chunk B = rest); each half is packed
    greedily into GH groups of <=128 dsts whose edges fit 1024 slots
    per source-chunk. Group blocks own static 128-row slices of the
    permuted layout.
  - The halo replica lives in HBM in **fp8e4** as two chunk tensors
    (hflA/hflB). Per layer the shard's two chunk halves are AllGathered
    SEPARATELY: chunk-A mid-layer (hidden behind chunk-B compute),
    chunk-B at layer end (hidden behind the next layer's chunk-A
    gathers). Sources are gathered per edge slot with 256B descriptors.
  - The root path (own shard) stays bf16 feat-major in SBUF, so only
    the neighbor-mean term sees fp8 quantization.
  - Gathers batch U=2 groups per dma_gather call (2048 descriptors =
    the 128/engine SWDGE ring limit) to amortize the ~1us emission
    fixed cost.
  - Segment-sum = one-hot fp8 matmuls (stationary stab block, moving
    gathered rows, 256-col streams); mean scale + bias/relu run on the
    otherwise-idle ScalarE, transposing copies on VectorE.
"""

import sys

sys.path.insert(0, "/opt/trn_rl_repo")

import numpy as np
import ml_dtypes

import concourse.bass as bass
import concourse.bacc as bacc
import concourse.tile as tile
import concourse.mybir as mybir
from concourse.bass_utils import run_bass_kernel_spmd

BF16 = ml_dtypes.bfloat16

N = 50000
E = 800000
D = 256
L = 3
P = 8
NSH = N // P            # 6250 nodes per core
HSH = NSH // 2          # 3125 nodes per core-half (chunk)
CAPB = 8                # gather blocks (of 128 slots) per src-chunk per group
CAP = CAPB * 128        # 1024 edge slots per src-chunk per group
NBLK = 2 * CAPB         # 16 segment blocks per group
U = 1                   # groups per gather call / per table-load unit


def _pack_idx16(idx):
    """Pack idx list (len multiple of 16) into [128, len/16] int16 layout:
    slot j -> [j % 16, j // 16], replicated to all 8 Q7-core stripes."""
    n = idx.shape[0]
    return np.tile(idx.reshape(n // 16, 16).T, (8, 1)).astype(np.int16)


def _preprocess(x, edge_index):
    """Group edges by dst windows per core-half; build permuted layout +
    gather/segment tables. Returns dict of host arrays + layout info."""
    src = edge_index[0].astype(np.int64)
    dst = edge_index[1].astype(np.int64)
    deg = np.bincount(dst, minlength=N).astype(np.float64)
    inv_deg = (1.0 / np.maximum(deg, 1.0)).astype(np.float32)

    # source chunk membership: position within the owner core's range
    srcB = (src % NSH) >= HSH

    halves = []     # [(core, half)] -> (groups, s_c, d_c, isB)
    for c in range(P):
        for h in range(2):
            lo = c * NSH + h * HSH
            hi = lo + HSH
            m = (dst >= lo) & (dst < hi)
            s_c = src[m]
            d_c = dst[m] - lo
            order = np.argsort(d_c, kind="stable")
            s_c, d_c = s_c[order], d_c[order]
            isB = srcB[np.nonzero(m)[0][order]]
            degA = np.bincount(d_c[~isB], minlength=HSH)
            degB = np.bincount(d_c[isB], minlength=HSH)
            assert degA.max() <= CAP and degB.max() <= CAP

            groups = []  # (base, end)
            base, ca, cb = 0, 0, 0
            for dd in range(HSH):
                da, db = degA[dd], degB[dd]
                if (ca + da > CAP) or (cb + db > CAP) or (dd - base >= 128):
                    groups.append((base, dd))
                    base, ca, cb = dd, 0, 0
                ca += da
                cb += db
            groups.append((base, HSH))
            halves.append((groups, s_c, d_c, isB))

    GH = max(len(hh[0]) for hh in halves)
    GH = ((GH + U - 1) // U) * U    # pad each half to unit multiple
    G = 2 * GH                      # groups per core
    GHP = GH * 128                  # permuted rows per core-half
    GP = G * 128                    # permuted rows per core
    NP = P * GP                     # total permuted rows
    NPH = NP // 2                   # rows per chunk tensor
    assert NPH < 32768

    # node id -> (shard-local permuted row, chunk row)
    perm = np.full(N, -1, dtype=np.int64)     # -> c*GP + (h*GH+g)*128 + r
    cperm = np.full(N, -1, dtype=np.int64)    # -> c*GHP + g*128 + r  (in chunk h)
    for c in range(P):
        for h in range(2):
            groups = halves[c * 2 + h][0]
            for g, (base, end) in enumerate(groups):
                span = end - base
                nid0 = c * NSH + h * HSH + base
                perm[nid0 : nid0 + span] = (
                    c * GP + (h * GH + g) * 128 + np.arange(span)
                )
                cperm[nid0 : nid0 + span] = (
                    c * GHP + g * 128 + np.arange(span)
                )
    assert (perm >= 0).all() and (cperm >= 0).all()

    gidxA = np.zeros((P, 128, G * CAP // 16), dtype=np.int16)
    gidxB = np.zeros((P, 128, G * CAP // 16), dtype=np.int16)
    s_all = np.zeros((P, 128, G * NBLK, 128), dtype=np.float32)
    invd_all = np.ones((P, 128, G), dtype=np.float32)
    for c in range(P):
        for h in range(2):
            groups, s_c, d_c, isB = halves[c * 2 + h]
            cs_c = cperm[s_c]
            eA = np.nonzero(~isB)[0]
            eB = np.nonzero(isB)[0]
            dA = d_c[eA]
            dB = d_c[eB]
            for g in range(GH):
                gg = h * GH + g       # group index within the core
                if g < len(groups):
                    base, end = groups[g]
                else:
                    base, end = 0, 0
                idxA = np.zeros(CAP, dtype=np.int16)
                idxB = np.zeros(CAP, dtype=np.int16)
                loA, hiA = np.searchsorted(dA, base), np.searchsorted(dA, end)
                loB, hiB = np.searchsorted(dB, base), np.searchsorted(dB, end)
                kA, kB = hiA - loA, hiB - loB
                assert kA <= CAP and kB <= CAP
                # ascending source order within the call -> better HBM
                # page locality for the random 256B descriptor reads
                srtA = np.argsort(cs_c[eA[loA:hiA]], kind="stable")
                srtB = np.argsort(cs_c[eB[loB:hiB]], kind="stable")
                idxA[:kA] = cs_c[eA[loA:hiA]][srtA]
                idxB[:kB] = cs_c[eB[loB:hiB]][srtB]
                cs = slice(gg * CAP // 16, (gg + 1) * CAP // 16)
                gidxA[c, :, cs] = _pack_idx16(idxA)
                gidxB[c, :, cs] = _pack_idx16(idxB)

                if g < len(groups):
                    invd_all[c, : end - base, gg] = inv_deg[
                        c * NSH + h * HSH + base : c * NSH + h * HSH + end
                    ]
                if kA:
                    jj = np.arange(kA)
                    dloc = (d_c[eA[loA:hiA]] - base)[srtA]
                    s_all[c, jj % 128, gg * NBLK + jj // 128, dloc] = 1.0
                if kB:
                    jj = np.arange(kB)
                    dloc = (d_c[eB[loB:hiB]] - base)[srtB]
                    s_all[c, jj % 128, gg * NBLK + CAPB + jj // 128, dloc] = 1.0

    return {
        "G": G,
        "perm": perm,
        "cperm": cperm,
        "gidxA": gidxA,
        "gidxB": gidxB,
        "stab": s_all.astype(mybir.dt.np(mybir.dt.float8e4)),
        "invd": invd_all,
    }


def _build_program(G, queue_map=None):
    """Build + compile the single SPMD program (parametrized by group count).

    queue_map: per-gather (emission order) SWDGE queue assignment. Tile
    binds each DMASW sem lane (scheduled-order round-robin over Pool DMA
    instructions, mod 8) permanently to one queue, so queue must equal
    the scheduled lane mod 4 — discovered via a first compile pass.
    Returns (nc, gather_instruction_names_in_emission_order).
    """
    GH = G // 2
    GHP = GH * 128
    GP = G * 128
    NP = P * GP
    NPH = NP // 2
    NU = G // U
    NUH = GH // U                   # units per chunk half
    nc = bacc.Bacc("TRN2", target_bir_lowering=False, debug=False, num_devices=P,
                   num_swdge_queues=4)
    f32, bf16, i16 = mybir.dt.float32, mybir.dt.bfloat16, mybir.dt.int16
    fp8 = mybir.dt.float8e4
    RELU = mybir.ActivationFunctionType.Relu
    IDENT = mybir.ActivationFunctionType.Identity

    xhA = nc.dram_tensor("xhA", [NPH, D], fp8, kind="ExternalInput")
    xhB = nc.dram_tensor("xhB", [NPH, D], fp8, kind="ExternalInput")
    xsT = nc.dram_tensor("xsT", [128, 2, GP], bf16, kind="ExternalInput")
    wl = nc.dram_tensor("wl", [L, 2, 128, D], bf16, kind="ExternalInput")
    wr = nc.dram_tensor("wr", [L, 2, 128, D], bf16, kind="ExternalInput")
    bias = nc.dram_tensor("bias", [L, 2, 128, 1], f32, kind="ExternalInput")
    ident = nc.dram_tensor("ident", [128, 128], bf16, kind="ExternalInput")
    gidxA = nc.dram_tensor("gidxA", [128, G * CAP // 16], i16, kind="ExternalInput")
    gidxB = nc.dram_tensor("gidxB", [128, G * CAP // 16], i16, kind="ExternalInput")
    stab = nc.dram_tensor("stab", [128, G * NBLK, 128], fp8, kind="ExternalInput")
    invd = nc.dram_tensor("invd", [128, G], f32, kind="ExternalInput")
    out = nc.dram_tensor("out", [GP, D], f32, kind="ExternalOutput")

    UCOL = U * CAP // 16        # gidx int16 columns per unit
    KPRE = 13                   # A-gather prefetch depth (units)
    gather_names = []

    with tile.TileContext(nc) as tc:
        with (
            tc.tile_pool(name="dram", bufs=1, space="DRAM") as dram,
            tc.tile_pool(name="const", bufs=1) as const,
            tc.tile_pool(name="xt", bufs=2) as xtp,
            tc.tile_pool(name="ga", bufs=KPRE + 2) as gap,
            tc.tile_pool(name="gb", bufs=6) as gbp,
            tc.tile_pool(name="sp", bufs=4) as sp,
            tc.tile_pool(name="gi", bufs=KPRE + 6) as gip,
            tc.tile_pool(name="stage", bufs=6) as stage,
            tc.tile_pool(name="pa", bufs=3, space="PSUM") as pap,
            tc.tile_pool(name="py", bufs=3, space="PSUM") as pyp,
            tc.tile_pool(name="pt", bufs=2, space="PSUM") as ptp,
        ):
            hsh_d = [
                dram.tile([GP, D], fp8, tag=f"hsh{i}", name=f"hsh{i}")
                for i in range(2)
            ]
            hflA_d = [
                dram.tile([NPH, D], fp8, tag=f"hflA{i}", name=f"hflA{i}",
                          addr_space="Shared")
                for i in range(2)
            ]
            hflB_d = [
                dram.tile([NPH, D], fp8, tag=f"hflB{i}", name=f"hflB{i}",
                          addr_space="Shared")
                for i in range(2)
            ]

            # resident constants
            w_sb = {}
            for l in range(L):
                for k in range(2):
                    t = const.tile([128, D], bf16, tag=f"wl{l}{k}", name=f"wl{l}{k}")
                    nc.sync.dma_start(t[:], wl[l, k])
                    w_sb[("l", l, k)] = t
                    t = const.tile([128, D], bf16, tag=f"wr{l}{k}", name=f"wr{l}{k}")
                    nc.sync.dma_start(t[:], wr[l, k])
                    w_sb[("r", l, k)] = t
            b_sb = {}
            for l in range(L):
                for mh in range(2):
                    t = const.tile([128, 1], f32, tag=f"b{l}{mh}", name=f"b{l}{mh}")
                    nc.sync.dma_start(t[:], bias[l, mh])
                    b_sb[(l, mh)] = t
            id_sb = const.tile([128, 128], bf16, tag="ident", name="id_sb")
            nc.sync.dma_start(id_sb[:], ident[:])
            iv_sb = const.tile([128, G], f32, tag="ivall", name="iv_sb")
            nc.sync.dma_start(iv_sb[:], invd[:])

            # persistent transposed-shard buffers (root path, feat-major)
            xt = [xtp.tile([128, 2, GP], bf16, tag="xt", name=f"xt{i}")
                  for i in range(2)]
            nc.sync.dma_start(xt[0][:], xsT[:])

            cur = 0
            for l in range(L):
                srcA_t = xhA if l == 0 else hflA_d[l - 1]
                srcB_t = xhB if l == 0 else hflB_d[l - 1]

                def emit_gatherA(u):
                    giA = gip.tile([128, UCOL], i16, name="giA", tag="gi")
                    nc.sync.dma_start(giA[:], gidxA[:, u * UCOL : (u + 1) * UCOL])
                    ga = gap.tile([128, U * CAPB, D], fp8, name="ga")
                    qa = queue_map[len(gather_names)] if queue_map else 0
                    gi_a = nc.gpsimd.dma_gather(
                        ga[:], srcA_t[:], giA[:],
                        U * CAP, U * CAP, D, queue_num=qa,
                    )
                    gather_names.append(gi_a.ins.name)
                    return ga

                # A-side prefetch: keeps the in-order Pool engine streaming
                # A gathers (which only need the mid-layer chunk-A
                # AllGather) while chunk-B's AllGather is still in flight.
                ga_q = [emit_gatherA(u) for u in range(min(KPRE, NU))]
                for u in range(NU):
                    if u + KPRE < NU:
                        ga_q.append(emit_gatherA(u + KPRE))
                    st = sp.tile([128, U * NBLK, 128], fp8, name="st")
                    nc.sync.dma_start(
                        st[:], stab[:, u * U * NBLK : (u + 1) * U * NBLK, :]
                    )
                    giB = gip.tile([128, UCOL], i16, name="giB", tag="gi")
                    nc.sync.dma_start(giB[:], gidxB[:, u * UCOL : (u + 1) * UCOL])
                    ga = ga_q[u]
                    gb = gbp.tile([128, U * CAPB, D], fp8, name="gb")
                    qb = queue_map[len(gather_names)] if queue_map else 0
                    gi_b = nc.gpsimd.dma_gather(
                        gb[:], srcB_t[:], giB[:],
                        U * CAP, U * CAP, D, queue_num=qb,
                    )
                    gather_names.append(gi_b.ins.name)

                    for j in range(U):
                        gg = u * U + j
                        gs = slice(gg * 128, (gg + 1) * 128)
                        # segment-sum: agg[dst_slot, feat] in PSUM (fp8 x fp8)
                        pa = pap.tile([128, D], f32, name="pa")
                        for q in range(CAPB):
                            nc.tensor.matmul(
                                pa[:],
                                st[:, j * NBLK + q, :],
                                ga[:, j * CAPB + q, :],
                                start=(q == 0),
                                stop=False,
                            )
                        for q in range(CAPB):
                            nc.tensor.matmul(
                                pa[:],
                                st[:, j * NBLK + CAPB + q, :],
                                gb[:, j * CAPB + q, :],
                                start=False,
                                stop=(q == CAPB - 1),
                            )
                        # mean scale (per-dst inv_deg) + downcast to bf16
                        ab = stage.tile([128, D], bf16, name="ab", tag="ab")
                        nc.scalar.activation(
                            ab[:], pa[:], IDENT, scale=iv_sb[:, gg : gg + 1]
                        )
                        # transpose agg to feat-major
                        aT = stage.tile([128, 2, 128], bf16, name="aT", tag="aT")
                        for k in range(2):
                            pt = ptp.tile([128, 128], bf16, name="pt")
                            nc.tensor.transpose(
                                pt[:], ab[:, k * 128 : (k + 1) * 128], id_sb[:]
                            )
                            nc.vector.tensor_copy(aT[:, k, :], pt[:])

                        # dense: yT[mh] = sum_k Wl[k,mh]^T aggT[k] + Wr[k,mh]^T xT[k]
                        py = pyp.tile([128, 2, 128], f32, name="py")
                        for mh in range(2):
                            ms = slice(mh * 128, (mh + 1) * 128)
                            nc.tensor.matmul(py[:, mh, :], w_sb[("l", l, 0)][:, ms],
                                             aT[:, 0, :], start=True, stop=False)
                            nc.tensor.matmul(py[:, mh, :], w_sb[("l", l, 1)][:, ms],
                                             aT[:, 1, :], start=False, stop=False)
                            nc.tensor.matmul(py[:, mh, :], w_sb[("r", l, 0)][:, ms],
                                             xt[cur][:, 0, gs], start=False, stop=False)
                            nc.tensor.matmul(py[:, mh, :], w_sb[("r", l, 1)][:, ms],
                                             xt[cur][:, 1, gs], start=False, stop=True)
                        if l < L - 1:
                            # bias+relu lands straight in the next layer's
                            # feat-major root buffer
                            yT_view = [xt[1 - cur][:, mh, gs] for mh in range(2)]
                        else:
                            yT = stage.tile([128, 2, 128], bf16, name="yT", tag="yT")
                            yT_view = [yT[:, mh, :] for mh in range(2)]
                        for mh in range(2):
                            nc.scalar.activation(
                                yT_view[mh], py[:, mh, :],
                                RELU if l < L - 1 else IDENT,
                                bias=b_sb[(l, mh)][:],
                            )
                        # back to row-major for the halo replica / output
                        ydt = fp8 if l < L - 1 else f32
                        yr = stage.tile([128, D], ydt, name="yr",
                                        tag=f"yr{l == L - 1}")
                        for mh in range(2):
                            pt2 = ptp.tile([128, 128], bf16, name="pt2", tag="pt")
                            nc.tensor.transpose(pt2[:], yT_view[mh], id_sb[:])
                            nc.vector.tensor_copy(
                                yr[:, mh * 128 : (mh + 1) * 128], pt2[:]
                            )
                        if l < L - 1:
                            nc.sync.dma_start(hsh_d[l][gs, :], yr[:])
                        else:
                            nc.sync.dma_start(out[gs, :], yr[:])

                    if l < L - 1 and u == NUH - 1:
                        # chunk-A AllGather launches mid-layer, overlapped
                        # with chunk-B compute
                        nc.gpsimd.collective_compute(
                            "AllGather",
                            mybir.AluOpType.bypass,
                            replica_groups=[list(range(P))],
                            ins=[hsh_d[l][0:GHP, :]],
                            outs=[hflA_d[l][:]],
                        )

                if l < L - 1:
                    nc.gpsimd.collective_compute(
                        "AllGather",
                        mybir.AluOpType.bypass,
                        replica_groups=[list(range(P))],
                        ins=[hsh_d[l][GHP : 2 * GHP, :]],
                        outs=[hflB_d[l][:]],
                    )
                    cur = 1 - cur

    nc.compile()
    return nc, gather_names


def _gather_lanes(nc, gather_names):
    """scheduled DMASW lane (0-7) per gather, keyed by instruction name."""
    lanes = {}
    for b in nc.m.functions[0].blocks:
        for i in b.instructions:
            if "Gather" in type(i).__name__:
                lanes[i.name] = i.bass_scheduled_proc - 11
    return [lanes[n] for n in gather_names]


_CACHE = {}


def _get_program(G):
    if G not in _CACHE:
        nc, names = _build_program(G)
        lanes = _gather_lanes(nc, names)
        for _ in range(4):
            qmap = [ln % 4 for ln in lanes]
            nc, names = _build_program(G, queue_map=qmap)
            lanes2 = _gather_lanes(nc, names)
            if lanes2 == lanes:
                break
            lanes = lanes2
        else:
            raise RuntimeError("SWDGE queue/lane fixpoint did not converge")
        _CACHE[G] = nc
    return _CACHE[G]


LAST_EXEC_NS = None


def kernel(x, edge_index, Wl, Wr, b, _trace=False):
    global LAST_EXEC_NS
    x = np.asarray(x, dtype=np.float32)
    edge_index = np.asarray(edge_index)
    Wl = np.asarray(Wl, dtype=np.float32)
    Wr = np.asarray(Wr, dtype=np.float32)
    b = np.asarray(b, dtype=np.float32)

    pre = _preprocess(x, edge_index)
    G = pre["G"]
    GP = G * 128
    NP = P * GP
    NPH = NP // 2
    nc = _get_program(G)

    FP8NP = mybir.dt.np(mybir.dt.float8e4)
    # permuted replicas (fp8, chunk layouts) for the layer-0 gather
    xh32 = np.zeros((NP, D), dtype=np.float32)
    xh32[pre["perm"]] = x
    # chunk membership: half h of node = (perm % GP) // (GP//2)
    half = (pre["perm"] % GP) // (GP // 2)
    xch = np.zeros((2, NPH, D), dtype=FP8NP)
    xf8 = x.astype(FP8NP)
    xch[half, pre["cperm"]] = xf8

    wl_h = np.ascontiguousarray(Wl.reshape(L, 2, 128, D).astype(BF16))
    wr_h = np.ascontiguousarray(Wr.reshape(L, 2, 128, D).astype(BF16))
    b_h = np.ascontiguousarray(b.reshape(L, 2, 128, 1).astype(np.float32))
    id_h = np.eye(128, dtype=BF16)

    in_maps = []
    for c in range(P):
        xs = xh32[c * GP : (c + 1) * GP]
        xsT = np.ascontiguousarray(
            xs.T.reshape(2, 128, GP).transpose(1, 0, 2).astype(BF16)
        )
        in_maps.append(
            {
                "xhA": xch[0],
                "xhB": xch[1],
                "xsT": xsT,
                "wl": wl_h,
                "wr": wr_h,
                "bias": b_h,
                "ident": id_h,
                "gidxA": pre["gidxA"][c],
                "gidxB": pre["gidxB"][c],
                "stab": pre["stab"][c],
                "invd": pre["invd"][c],
            }
        )

    res = run_bass_kernel_spmd(
        nc, in_maps, core_ids=list(range(P)), trace=bool(_trace)
    )
    LAST_EXEC_NS = res.exec_time_ns

    out_full = np.empty((N, D), dtype=np.float32)
    outs = np.concatenate([res.results[c]["out"] for c in range(P)], axis=0)
    out_full[:] = outs[pre["perm"]]
    return out_full

